# revision 39
# baseline (speedup 1.0000x reference)
"""Single-head attention (B=8, S=2048, D=U=1024) on 8 TRN2 NeuronCores.

Sharding: data-parallel over batch — core b computes batch b end-to-end,
no cross-core communication.

Per-core pipeline (fp32 PSUM accumulation everywhere):
  A. x [S,D] f32 --SWDGE cast--> x_sb [s,d] bf16 in SBUF (16 s-tiles,
     two half-tile DMAs each), then PE-transposed 128x128 blocks
     (is_transpose matmul vs identity) -> PSUM -> DVE/ACT copies ->
     xT [D,S] bf16.  This replaces the old DRAM-bounce + 32 serialized
     xbar-DMA transposes (~40us of PE idle); the PE starts transposing
     as soon as the first half-tile lands, with the first Q projection
     chunks interleaved right behind, and ~25 dummy identity matmuls up
     front keep the HAM clock gate from starting the real work at the
     cold 1.2 GHz p-state.
  B. W* f32 --SWDGE cast--> SBUF bf16 half-width tiles (3-slot ring).
     SWDGE queue order: x0, Wq.0 (u-sliced), x1-15, Wq.1, Wk.*, Wv.*
     — each arrives just before its consumer.
  C. Qt = Wq^T xT + bq  [U,S] fp8e4m3   (no 1/sqrt(U) here — folded into
     the exp scale in D so Q keeps fp8-friendly magnitude)
     Kt = Wk^T xT + bk  [U,S] fp8e4m3
     V  = xT^T Wv + bv  [S,U] bf16, with a ones column appended at u=1024
     (feeds the softmax denominator in E for free).
  D. scores^T[k,q] = sum_u Kt[u,k] Qt[u,q] via fp8 DoubleRow matmuls
     (2 u-tiles per instruction, 2x FLOP rate); the padding mask adds the
     rank-1 term c_k*m_q (c = -320000*(1-m), pre-scaled for the exp scale)
     via one DVE scalar_tensor_tensor per PSUM tile; Et = exp(scores^T/32)
     on ACT (scale=1/32), PSUM->SBUF bf16.  No max-subtraction: scores are
     O(1) and masked entries underflow to exactly 0, matching fp32 ref.
  E. ctx[q,u'] = sum_k Et[k,q]^T V[k,u']  over the 1025-wide V (u' = u plus
     the ones column) in a 342/342/341 column split (each <= one PSUM bank,
     no 1-column denominator matmuls); out = ctx * (1/denom) in the
     PSUM->SBUF epilogue (per-partition scalar), denom = ctx[:,1024].

Numerics: fp8 is used ONLY for the scores contraction (Q,K operands).
Simulated end-to-end rel err 1.80e-2 (threshold 2e-2); bf16 everywhere
else keeps the PV path at bf16 accuracy.  exp underflow handles masking
exactly; for m_q=0 rows softmax shift-invariance matches the reference.
"""

import os
import sys

import numpy as np

for _p in ("/opt/trn_rl_repo", "/opt/pypackages"):
    if _p not in sys.path and os.path.isdir(_p):
        sys.path.append(_p)

import concourse.bass as bass
import concourse.tile as tile
from concourse import bacc, mybir
from concourse.bass import ts
from concourse.bass_utils import run_bass_kernel_spmd
from concourse.masks import make_identity

P = 128
B, S, D, U = 8, 2048, 1024, 1024
NCORES = 8
NG = 512  # matmul moving free dim (one fp32 PSUM bank)
DT, UT, ST, KT = D // P, U // P, S // P, S // P  # 8, 8, 16, 16
SG, QG = S // NG, S // NG  # 4, 4
UG = U // NG  # 2
UH = UT // 2  # u-tiles per W half
VW = 1028  # v_sb row width: 1024 v cols + ones col at 1024 + pad to 8B
# phase-E 3-way split of the 1025 live v columns (each chunk <= 512)
ESPLIT = (342, 342, 341)

F32 = mybir.dt.float32
BF16 = mybir.dt.bfloat16
FP8 = mybir.dt.float8e4
I32 = mybir.dt.int32
AF = mybir.ActivationFunctionType
ALU = mybir.AluOpType
DR = mybir.MatmulPerfMode.DoubleRow

_cache = {}
last_results = None


def _emit(tc):
    nc = tc.nc
    x_d = nc.dram_tensor("x", [S, D], F32, kind="ExternalInput").ap()
    m_d = nc.dram_tensor("mask", [1, S], I32, kind="ExternalInput").ap()
    w_d = {
        "q": nc.dram_tensor("wq", [D, U], F32, kind="ExternalInput").ap(),
        "k": nc.dram_tensor("wk", [D, U], F32, kind="ExternalInput").ap(),
        "v": nc.dram_tensor("wv", [D, U], F32, kind="ExternalInput").ap(),
    }
    bq_d = nc.dram_tensor("bq", [1, U], F32, kind="ExternalInput").ap()
    bk_d = nc.dram_tensor("bk", [1, U], F32, kind="ExternalInput").ap()
    bv_d = nc.dram_tensor("bv", [1, U], F32, kind="ExternalInput").ap()
    out_d = nc.dram_tensor("out", [S, U], F32, kind="ExternalOutput").ap()

    # ---------------- small persistent tensors ----------------
    consts, free_consts = tc.tile(shape=[P, 2 * UT + KT], dtype=F32, name="consts")
    bq_cols = consts[:, 0:UT]
    bk_cols = consts[:, UT : 2 * UT]
    c_cols = consts[:, 2 * UT : 2 * UT + KT]  # -320000*(1-m), per k partition

    rows, free_rows = tc.tile(shape=[1, S + U + P], dtype=BF16, name="rows")
    m_row = rows[:, 0:S]
    bv_row = rows[:, S : S + U]
    ones_row = rows[:, S + U : S + U + P]

    ident, free_ident = tc.tile(shape=[P, P], dtype=BF16, name="ident")
    m_bcast, free_m_bcast = tc.tile(shape=[P, S], dtype=BF16, name="m_bcast")
    bv_bcast, free_bv_bcast = tc.tile(shape=[P, U], dtype=BF16, name="bv_bcast")

    make_identity(nc, ident[:])

    with tc.tile_pool(name="big", bufs=1) as big:

        def load_w_half(which, half, sliced=False):
            # sliced: one cast-DMA per 128-col u-slice so each proj_chunk
            # u4-iteration can start as soon as its own slice lands.
            wt = big.tile([P, DT, NG], BF16, tag="w", bufs=3, name=f"w{which}_{half}")
            src = w_d[which].rearrange("(t p) u -> p t u", p=P)[:, :, ts(half, NG)]
            if sliced:
                for u4 in range(UH):
                    nc.gpsimd.dma_start(wt[:, :, ts(u4, P)], src[:, :, ts(u4, P)])
            else:
                nc.gpsimd.dma_start(wt[:], src)  # f32 -> bf16 cast (SWDGE)
            return wt

        # x [s,d] -> SBUF bf16 via SWDGE cast, two half-tile DMAs per
        # 128-row s-tile so transposes (and the projection chains behind
        # them) start on the first half while the second is in flight.
        # x_sb lives in the 64KB slotA tag; et_sb takes the slot over in
        # phase D (after the last transpose read).
        x_sb = big.tile([P, ST, D], BF16, tag="slotA", name="x_sb")
        x_src = x_d.rearrange("(t p) d -> p t d", p=P)
        DH = D // 2

        def load_x(st):
            for h in range(2):
                nc.gpsimd.dma_start(
                    x_sb[:, st, ts(h, DH)], x_src[:, st, ts(h, DH)]
                )

        # small HWDGE loads first on the sync queue (a few KB)
        m_i32 = big.tile([1, S], I32, tag="qt", name="m_i32")
        nc.sync.dma_start(m_i32[:], m_d)
        mk_i32 = big.tile([P, KT], I32, tag="v", name="mk_i32")
        nc.sync.dma_start(mk_i32[:], m_d.rearrange("a (t p) -> p (a t)", p=P))
        bv_f32 = big.tile([1, U], F32, tag="kt", name="bv_f32")
        nc.sync.dma_start(bv_f32[:], bv_d)
        nc.sync.dma_start(bq_cols, bq_d.rearrange("a (j p) -> p (a j)", p=P))
        nc.sync.dma_start(bk_cols, bk_d.rearrange("a (j p) -> p (a j)", p=P))

        for st in range(4):
            load_x(st)
        wq_h = [load_w_half("q", 0, sliced=True)]
        for st in range(4, ST):
            load_x(st)
        wq_h.append(load_w_half("q", 1))

        nc.vector.memset(ones_row, 1.0)
        nc.vector.tensor_copy(m_row, m_i32[:])
        # c = m*320000 - 320000 -> 0 where m==1, -320000 where m==0
        # (exp applies scale=1/32, so this is -10000 in score units)
        nc.vector.tensor_scalar(
            c_cols, mk_i32[:], 320000.0, -320000.0, ALU.mult, ALU.add
        )
        nc.vector.tensor_copy(bv_row, bv_f32[:])

        # broadcast m and bv across partitions via ones-column matmuls
        with tc.tile_pool(name="psInit", bufs=2, space="PSUM") as psInit:
            # HAM pre-warm: the PE clock gate defaults to 1.2 GHz and needs
            # ~3.4us of sustained activity to open to 2.4 GHz.  These dummy
            # matmuls run while the first x tile is still in flight (PE
            # would idle anyway), so the real work starts at full clock.
            # 4-tile ring so consecutive dummies pipeline (a single tile
            # WAW-serializes at ~313ns each with drain gaps the HAM counts
            # as idle; the ring keeps the busy window contiguous)
            wps = [
                psInit.tile([P, P], F32, tag=f"warm{j}", bufs=1, name=f"warm_ps{j}")
                for j in range(4)
            ]
            for i in range(48):
                nc.tensor.matmul(wps[i % 4][:], lhsT=ident[:], rhs=ident[:])
            for qg in range(QG):
                pi = psInit.tile([P, NG], F32, tag="init", name="ps_init")
                nc.tensor.matmul(
                    pi[:], lhsT=ones_row[:, 0:P], rhs=m_row[:, ts(qg, NG)]
                )
                nc.vector.tensor_copy(m_bcast[:, ts(qg, NG)], pi[:])
            for ug in range(UG):
                pi = psInit.tile([P, NG], F32, tag="init", name="ps_init2")
                nc.tensor.matmul(
                    pi[:], lhsT=ones_row[:, 0:P], rhs=bv_row[:, ts(ug, NG)]
                )
                nc.vector.tensor_copy(bv_bcast[:, ts(ug, NG)], pi[:])

        # ---------------- phases A+C interleaved ----------------
        xT = big.tile([P, DT, S], BF16, tag="xT", name="xT")
        qt_sb = big.tile([P, UT, S], FP8, tag="qt", name="qt_sb")
        kt_sb = big.tile([P, UT, S], FP8, tag="kt", name="kt_sb")
        v_sb = big.tile([P, ST, VW], BF16, tag="v", name="v_sb")
        nc.vector.memset(v_sb[:, :, U : U + 1], 1.0)  # denominator ones column

        with (
            tc.tile_pool(name="psT", bufs=2, space="PSUM") as psT,
            tc.tile_pool(name="psC", bufs=6, space="PSUM") as psC,
        ):
            copy_engines = (nc.vector, nc.scalar)
            _copy_rr = [0]

            def copy_out(dst, src):
                eng = copy_engines[_copy_rr[0] % 2]
                _copy_rr[0] += 1
                if eng is nc.scalar:
                    eng.copy(dst, src)
                else:
                    eng.tensor_copy(dst, src)

            def fill(n):
                # tiny dummy matmuls between early DMA-paced transpose
                # groups: keep the HAM activity window busy so the PE is
                # not re-throttled to 1.2 GHz while x tiles trickle in
                wf = psT.tile([P, P], F32, tag="t", name="fill_ps")
                for _ in range(n):
                    nc.tensor.matmul(wf[:], lhsT=ident[:], rhs=ident[:])

            def transpose_st(st):
                # x tile [s128, d1024] -> xT[:, :, st*128] [d, s128],
                # in d-halves matching the half-tile x DMAs
                for h in range(2):
                    pt = psT.tile([P, DT // 2, P], BF16, tag="t", name="ps_t")
                    for j in range(DT // 2):
                        dt = h * (DT // 2) + j
                        nc.tensor.transpose(
                            pt[:, j, :], x_sb[:, st, ts(dt, P)], ident[:]
                        )
                    copy_out(
                        xT[:, h * (DT // 2) : (h + 1) * (DT // 2), ts(st, P)], pt[:]
                    )
                if 1 <= st <= 5:
                    fill(8)

            def proj_chunk(which, half, sg, lo=0, width=NG):
                # Q^T / K^T: [u,s] = sum_d W[d,u] * xT[d,s].
                # lo/width select a sub-range of the sg column group so the
                # x-DMA-paced head can issue work at 2-s-tile granularity.
                w_h = wq_h[half] if which == "q" else wk_h[half]
                dst = qt_sb if which == "q" else kt_sb
                bias_cols = bq_cols if which == "q" else bk_cols
                base = sg * NG + lo
                for u4 in range(UH):
                    ut = half * UH + u4
                    ps = psC.tile([P, NG], F32, tag="proj", name="ps_proj")
                    for dt in range(DT):
                        nc.tensor.matmul(
                            ps[:, 0:width],
                            lhsT=w_h[:, dt, ts(u4, P)],
                            rhs=xT[:, dt, base : base + width],
                            start=(dt == 0),
                            stop=(dt == DT - 1),
                        )
                    nc.vector.tensor_scalar_add(
                        dst[:, ut, base : base + width],
                        ps[:, 0:width],
                        bias_cols[:, ut : ut + 1],
                    )

            # Q half-0 interleaved with the transposes at 2-s-tile (256 col)
            # granularity: each half-chunk only needs the two s-tiles just
            # transposed, so the PE issues projection work while the next
            # x tiles are still in flight.
            HW2 = NG // 2
            for sg in range(SG):
                for h in range(2):
                    transpose_st(4 * sg + 2 * h)
                    transpose_st(4 * sg + 2 * h + 1)
                    proj_chunk("q", 0, sg, lo=h * HW2, width=HW2)
            for sg in range(SG):
                proj_chunk("q", 1, sg)
            wk_h = [load_w_half("k", 0), load_w_half("k", 1)]
            for half in range(2):
                for sg in range(SG):
                    proj_chunk("k", half, sg)

            # V: [s,u] = sum_d xT[d,s] * Wv[d,u]; bv added in the epilogue
            for ug in range(UG):
                wv_h = load_w_half("v", ug)
                for st in range(ST):
                    pv = psC.tile([P, NG], F32, tag="proj", name="ps_v")
                    for dt in range(DT):
                        nc.tensor.matmul(
                            pv[:],
                            lhsT=xT[:, dt, ts(st, P)],
                            rhs=wv_h[:, dt, :],
                            start=(dt == 0),
                            stop=(dt == DT - 1),
                        )
                    nc.vector.tensor_tensor(
                        v_sb[:, st, ts(ug, NG)],
                        pv[:],
                        bv_bcast[:, ts(ug, NG)],
                        ALU.add,
                    )

        # ---------------- phase D: scores^T + mask + exp ----------------
        # fp8 DoubleRow: each matmul contracts two u-tiles (256 rows).
        et_sb = big.tile([P, KT, S], BF16, tag="slotA", name="et_sb")
        with tc.tile_pool(name="psD", bufs=8, space="PSUM") as psD:
            for kt in range(KT):
                pss = [
                    psD.tile([P, NG], F32, tag="sc", name="ps_sc") for _ in range(QG)
                ]
                for t in range(UT // 2):
                    for qg in range(QG):
                        nc.tensor.matmul(
                            pss[qg][:],
                            lhsT=kt_sb[:, 2 * t : 2 * t + 2, ts(kt, P)],
                            rhs=qt_sb[:, 2 * t : 2 * t + 2, ts(qg, NG)],
                            start=(t == 0),
                            stop=(t == UT // 2 - 1),
                            perf_mode=DR,
                        )
                for qg in range(QG):
                    # scores += c_k * m_q  (rank-1 mask term, on DVE —
                    # GPSIMD cannot access PSUM)
                    nc.vector.scalar_tensor_tensor(
                        pss[qg][:],
                        m_bcast[:, ts(qg, NG)],
                        c_cols[:, kt : kt + 1],
                        pss[qg][:],
                        ALU.mult,
                        ALU.add,
                    )
                    # Et = exp(scores/32); 1/sqrt(U) folded in here
                    nc.scalar.activation(
                        et_sb[:, kt, ts(qg, NG)], pss[qg][:], AF.Exp, scale=1.0 / 32.0
                    )

        # ---------------- phase E: PV(+denom column) + normalize ----------------
        e_off = (0, ESPLIT[0], ESPLIT[0] + ESPLIT[1])
        with tc.tile_pool(name="psE", bufs=6, space="PSUM") as psE:
            for qt in range(KT):
                pc = [
                    psE.tile([P, NG], F32, tag="ctx", name="ps_ctx") for _ in range(3)
                ]
                for kt in range(KT):
                    lhsT = et_sb[:, kt, ts(qt, P)]
                    first, last = kt == 0, kt == KT - 1
                    for j in range(3):
                        nc.tensor.matmul(
                            pc[j][:, 0 : ESPLIT[j]],
                            lhsT=lhsT,
                            rhs=v_sb[:, kt, e_off[j] : e_off[j] + ESPLIT[j]],
                            start=first,
                            stop=last,
                        )
                recip = big.tile([P, 1], F32, tag="kt", name="recip")
                # denominator = ones-column result: last col of chunk 2
                nc.vector.reciprocal(recip[:], pc[2][:, ESPLIT[2] - 1 : ESPLIT[2]])
                # per-chunk normalize + store so the final store starts as
                # early as possible (trims the kernel tail)
                o = big.tile([P, U], F32, tag="qt", name="o_sb")
                for j, w in ((0, ESPLIT[0]), (1, ESPLIT[1]), (2, ESPLIT[2] - 1)):
                    lo = e_off[j]
                    if j == 1:
                        # middle chunk on the (otherwise idle) ACT engine so
                        # the epilogue drains in parallel with the DVE chunks
                        nc.scalar.mul(o[:, lo : lo + w], pc[j][:, 0:w], recip[:])
                    else:
                        nc.vector.tensor_scalar_mul(
                            o[:, lo : lo + w], pc[j][:, 0:w], recip[:]
                        )
                    nc.sync.dma_start(
                        out_d[ts(qt, P), lo : lo + w], o[:, lo : lo + w]
                    )

    free_bv_bcast()
    free_m_bcast()
    free_ident()
    free_rows()
    free_consts()


def _build():
    if "nc" in _cache:
        return _cache["nc"]
    nc = bacc.Bacc("TRN2", target_bir_lowering=False, debug=False, num_devices=NCORES)
    with tile.TileContext(nc) as tc:
        _emit(tc)
    nc.compile()
    _cache["nc"] = nc
    return nc


def kernel(x, mask, Wq, bq, Wk, bk, Wv, bv):
    global last_results
    nc = _build()
    wq = np.ascontiguousarray(Wq, dtype=np.float32)
    wk = np.ascontiguousarray(Wk, dtype=np.float32)
    wv = np.ascontiguousarray(Wv, dtype=np.float32)
    bqr = np.ascontiguousarray(bq, dtype=np.float32).reshape(1, U)
    bkr = np.ascontiguousarray(bk, dtype=np.float32).reshape(1, U)
    bvr = np.ascontiguousarray(bv, dtype=np.float32).reshape(1, U)
    in_maps = []
    for b in range(B):
        in_maps.append(
            {
                "x": np.ascontiguousarray(x[b], dtype=np.float32),
                "mask": np.ascontiguousarray(mask[b], dtype=np.int32).reshape(1, S),
                "wq": wq,
                "wk": wk,
                "wv": wv,
                "bq": bqr,
                "bk": bkr,
                "bv": bvr,
            }
        )
    res = run_bass_kernel_spmd(
        nc,
        in_maps,
        core_ids=list(range(NCORES)),
        trace=bool(int(os.environ.get("KERNEL_TRACE", "0"))),
        tmpdir=os.environ.get("KERNEL_TRACE_DIR"),
    )
    last_results = res
    return np.stack([res.results[b]["out"] for b in range(B)])


# revision 40
# speedup vs baseline: 1.1866x; 1.1866x over previous
"""Single-head attention (B=8, S=2048, D=U=1024) on 8 TRN2 NeuronCores.

Sharding: data-parallel over batch — core b computes batch b end-to-end,
no cross-core communication.

Per-core pipeline (fp32 PSUM accumulation everywhere):
  A. x [S,D] f32 --SWDGE cast--> x_sb [s,d] bf16 in SBUF (16 s-tiles,
     two half-tile DMAs each), then PE-transposed 128x128 blocks
     (is_transpose matmul vs identity) -> PSUM -> DVE/ACT copies ->
     xT [D,S] bf16.  This replaces the old DRAM-bounce + 32 serialized
     xbar-DMA transposes (~40us of PE idle); the PE starts transposing
     as soon as the first half-tile lands, with the first Q projection
     chunks interleaved right behind, and ~25 dummy identity matmuls up
     front keep the HAM clock gate from starting the real work at the
     cold 1.2 GHz p-state.
  B. W* f32 --SWDGE cast--> SBUF bf16 half-width tiles (3-slot ring).
     SWDGE queue order: x0, Wq.0 (u-sliced), x1-15, Wq.1, Wk.*, Wv.*
     — each arrives just before its consumer.
  C. Qt = Wq^T xT + bq  [U,S] fp8e4m3   (no 1/sqrt(U) here — folded into
     the exp scale in D so Q keeps fp8-friendly magnitude)
     Kt = Wk^T xT + bk  [U,S] fp8e4m3
     V  = xT^T Wv + bv  [S,U] bf16, with a ones column appended at u=1024
     (feeds the softmax denominator in E for free).
  D. scores^T[k,q] = sum_u Kt[u,k] Qt[u,q] via fp8 DoubleRow matmuls
     (2 u-tiles per instruction, 2x FLOP rate); the padding mask adds the
     rank-1 term c_k*m_q (c = -320000*(1-m), pre-scaled for the exp scale)
     via one DVE scalar_tensor_tensor per PSUM tile; Et = exp(scores^T/32)
     on ACT (scale=1/32), PSUM->SBUF bf16.  No max-subtraction: scores are
     O(1) and masked entries underflow to exactly 0, matching fp32 ref.
  E. ctx[q,u'] = sum_k Et[k,q]^T V[k,u']  over the 1025-wide V (u' = u plus
     the ones column) in a 342/342/341 column split (each <= one PSUM bank,
     no 1-column denominator matmuls); out = ctx * (1/denom) in the
     PSUM->SBUF epilogue (per-partition scalar), denom = ctx[:,1024].

Numerics: fp8 is used ONLY for the scores contraction (Q,K operands).
Simulated end-to-end rel err 1.80e-2 (threshold 2e-2); bf16 everywhere
else keeps the PV path at bf16 accuracy.  exp underflow handles masking
exactly; for m_q=0 rows softmax shift-invariance matches the reference.
"""

import os
import sys

import numpy as np

for _p in ("/opt/trn_rl_repo", "/opt/pypackages"):
    if _p not in sys.path and os.path.isdir(_p):
        sys.path.append(_p)

import concourse.bass as bass
import concourse.tile as tile
from concourse import bacc, mybir
from concourse.bass import ts
from concourse.bass_utils import run_bass_kernel_spmd
from concourse.masks import make_identity

P = 128
B, S, D, U = 8, 2048, 1024, 1024
NCORES = 8
NG = 512  # matmul moving free dim (one fp32 PSUM bank)
DT, UT, ST, KT = D // P, U // P, S // P, S // P  # 8, 8, 16, 16
SG, QG = S // NG, S // NG  # 4, 4
UG = U // NG  # 2
UH = UT // 2  # u-tiles per W half
VW = 1028  # v_sb row width: 1024 v cols + ones col at 1024 + pad to 8B
# phase-E 3-way split of the 1025 live v columns (each chunk <= 512)
ESPLIT = (342, 342, 341)

F32 = mybir.dt.float32
BF16 = mybir.dt.bfloat16
FP8 = mybir.dt.float8e4
I32 = mybir.dt.int32
AF = mybir.ActivationFunctionType
ALU = mybir.AluOpType
DR = mybir.MatmulPerfMode.DoubleRow

_cache = {}
last_results = None


def _emit(tc):
    nc = tc.nc
    x_d = nc.dram_tensor("x", [S, D], F32, kind="ExternalInput").ap()
    m_d = nc.dram_tensor("mask", [1, S], I32, kind="ExternalInput").ap()
    w_d = {
        "q": nc.dram_tensor("wq", [D, U], F32, kind="ExternalInput").ap(),
        "k": nc.dram_tensor("wk", [D, U], F32, kind="ExternalInput").ap(),
        "v": nc.dram_tensor("wv", [D, U], F32, kind="ExternalInput").ap(),
    }
    bq_d = nc.dram_tensor("bq", [1, U], F32, kind="ExternalInput").ap()
    bk_d = nc.dram_tensor("bk", [1, U], F32, kind="ExternalInput").ap()
    bv_d = nc.dram_tensor("bv", [1, U], F32, kind="ExternalInput").ap()
    out_d = nc.dram_tensor("out", [S, U], F32, kind="ExternalOutput").ap()

    # ---------------- small persistent tensors ----------------
    consts, free_consts = tc.tile(shape=[P, 2 * UT + KT], dtype=F32, name="consts")
    bq_cols = consts[:, 0:UT]
    bk_cols = consts[:, UT : 2 * UT]
    c_cols = consts[:, 2 * UT : 2 * UT + KT]  # -320000*(1-m), per k partition

    rows, free_rows = tc.tile(shape=[1, S + U + P], dtype=BF16, name="rows")
    m_row = rows[:, 0:S]
    bv_row = rows[:, S : S + U]
    ones_row = rows[:, S + U : S + U + P]

    ident, free_ident = tc.tile(shape=[P, P], dtype=BF16, name="ident")
    m_bcast, free_m_bcast = tc.tile(shape=[P, S], dtype=BF16, name="m_bcast")
    bv_bcast, free_bv_bcast = tc.tile(shape=[P, U], dtype=BF16, name="bv_bcast")

    make_identity(nc, ident[:])

    with tc.tile_pool(name="big", bufs=1) as big:

        def load_w_half(which, half, sliced=False):
            # sliced: one cast-DMA per 128-col u-slice so each proj_chunk
            # u4-iteration can start as soon as its own slice lands.
            wt = big.tile([P, DT, NG], BF16, tag="w", bufs=3, name=f"w{which}_{half}")
            src = w_d[which].rearrange("(t p) u -> p t u", p=P)[:, :, ts(half, NG)]
            if sliced:
                for u4 in range(UH):
                    nc.gpsimd.dma_start(wt[:, :, ts(u4, P)], src[:, :, ts(u4, P)])
            else:
                nc.gpsimd.dma_start(wt[:], src)  # f32 -> bf16 cast (SWDGE)
            return wt

        # x [s,d] -> SBUF bf16 via SWDGE cast, two half-tile DMAs per
        # 128-row s-tile so transposes (and the projection chains behind
        # them) start on the first half while the second is in flight.
        # x_sb lives in the 64KB slotA tag; et_sb takes the slot over in
        # phase D (after the last transpose read).
        x_sb = big.tile([P, ST, D], BF16, tag="slotA", name="x_sb")
        x_src = x_d.rearrange("(t p) d -> p t d", p=P)
        DH = D // 2

        def load_x(st):
            for h in range(2):
                nc.gpsimd.dma_start(
                    x_sb[:, st, ts(h, DH)], x_src[:, st, ts(h, DH)]
                )

        # small HWDGE loads first on the sync queue (a few KB)
        m_i32 = big.tile([1, S], I32, tag="qt", name="m_i32")
        nc.sync.dma_start(m_i32[:], m_d)
        mk_i32 = big.tile([P, KT], I32, tag="v", name="mk_i32")
        nc.sync.dma_start(mk_i32[:], m_d.rearrange("a (t p) -> p (a t)", p=P))
        bv_f32 = big.tile([1, U], F32, tag="kt", name="bv_f32")
        nc.sync.dma_start(bv_f32[:], bv_d)
        nc.sync.dma_start(bq_cols, bq_d.rearrange("a (j p) -> p (a j)", p=P))
        nc.sync.dma_start(bk_cols, bk_d.rearrange("a (j p) -> p (a j)", p=P))

        for st in range(4):
            load_x(st)
        wq_h = [load_w_half("q", 0, sliced=True)]
        for st in range(4, ST):
            load_x(st)
        wq_h.append(load_w_half("q", 1))

        nc.vector.memset(ones_row, 1.0)
        nc.vector.tensor_copy(m_row, m_i32[:])
        # c = m*320000 - 320000 -> 0 where m==1, -320000 where m==0
        # (exp applies scale=1/32, so this is -10000 in score units)
        nc.vector.tensor_scalar(
            c_cols, mk_i32[:], 320000.0, -320000.0, ALU.mult, ALU.add
        )
        nc.vector.tensor_copy(bv_row, bv_f32[:])

        # broadcast m and bv across partitions via ones-column matmuls
        with tc.tile_pool(name="psInit", bufs=2, space="PSUM") as psInit:
            # HAM pre-warm: the PE clock gate defaults to 1.2 GHz and needs
            # ~3.4us of sustained activity to open to 2.4 GHz.  These dummy
            # matmuls run while the first x tile is still in flight (PE
            # would idle anyway), so the real work starts at full clock.
            # 4-tile ring so consecutive dummies pipeline (a single tile
            # WAW-serializes at ~313ns each with drain gaps the HAM counts
            # as idle; the ring keeps the busy window contiguous)
            wps = [
                psInit.tile([P, P], F32, tag=f"warm{j}", bufs=1, name=f"warm_ps{j}")
                for j in range(4)
            ]
            for i in range(48):
                nc.tensor.matmul(wps[i % 4][:], lhsT=ident[:], rhs=ident[:])
            for qg in range(QG):
                pi = psInit.tile([P, NG], F32, tag="init", name="ps_init")
                nc.tensor.matmul(
                    pi[:], lhsT=ones_row[:, 0:P], rhs=m_row[:, ts(qg, NG)]
                )
                nc.vector.tensor_copy(m_bcast[:, ts(qg, NG)], pi[:])
            for ug in range(UG):
                pi = psInit.tile([P, NG], F32, tag="init", name="ps_init2")
                nc.tensor.matmul(
                    pi[:], lhsT=ones_row[:, 0:P], rhs=bv_row[:, ts(ug, NG)]
                )
                nc.vector.tensor_copy(bv_bcast[:, ts(ug, NG)], pi[:])

        # ---------------- phases A+C interleaved ----------------
        xT = big.tile([P, DT, S], BF16, tag="xT", name="xT")
        qt_sb = big.tile([P, UT, S], FP8, tag="qt", name="qt_sb")
        kt_sb = big.tile([P, UT, S], FP8, tag="kt", name="kt_sb")
        v_sb = big.tile([P, ST, VW], BF16, tag="v", name="v_sb")
        nc.vector.memset(v_sb[:, :, U : U + 1], 1.0)  # denominator ones column

        with (
            tc.tile_pool(name="psT", bufs=2, space="PSUM") as psT,
            tc.tile_pool(name="psC", bufs=6, space="PSUM") as psC,
        ):
            copy_engines = (nc.vector, nc.scalar)
            _copy_rr = [0]

            def copy_out(dst, src):
                eng = copy_engines[_copy_rr[0] % 2]
                _copy_rr[0] += 1
                if eng is nc.scalar:
                    eng.copy(dst, src)
                else:
                    eng.tensor_copy(dst, src)

            def fill(n):
                # tiny dummy matmuls between early DMA-paced transpose
                # groups: keep the HAM activity window busy so the PE is
                # not re-throttled to 1.2 GHz while x tiles trickle in
                wf = psT.tile([P, P], F32, tag="t", name="fill_ps")
                for _ in range(n):
                    nc.tensor.matmul(wf[:], lhsT=ident[:], rhs=ident[:])

            def transpose_st(st):
                # x tile [s128, d1024] -> xT[:, :, st*128] [d, s128],
                # in d-halves matching the half-tile x DMAs
                for h in range(2):
                    pt = psT.tile([P, DT // 2, P], BF16, tag="t", name="ps_t")
                    for j in range(DT // 2):
                        dt = h * (DT // 2) + j
                        nc.tensor.transpose(
                            pt[:, j, :], x_sb[:, st, ts(dt, P)], ident[:]
                        )
                    copy_out(
                        xT[:, h * (DT // 2) : (h + 1) * (DT // 2), ts(st, P)], pt[:]
                    )
                if 1 <= st <= 3:
                    fill(4)

            def proj_chunk(which, half, sg, lo=0, width=NG):
                # Q^T / K^T: [u,s] = sum_d W[d,u] * xT[d,s].
                # lo/width select a sub-range of the sg column group so the
                # x-DMA-paced head can issue work at 2-s-tile granularity.
                w_h = wq_h[half] if which == "q" else wk_h[half]
                dst = qt_sb if which == "q" else kt_sb
                bias_cols = bq_cols if which == "q" else bk_cols
                base = sg * NG + lo
                for u4 in range(UH):
                    ut = half * UH + u4
                    ps = psC.tile([P, NG], F32, tag="proj", name="ps_proj")
                    for dt in range(DT):
                        nc.tensor.matmul(
                            ps[:, 0:width],
                            lhsT=w_h[:, dt, ts(u4, P)],
                            rhs=xT[:, dt, base : base + width],
                            start=(dt == 0),
                            stop=(dt == DT - 1),
                        )
                    nc.vector.tensor_scalar_add(
                        dst[:, ut, base : base + width],
                        ps[:, 0:width],
                        bias_cols[:, ut : ut + 1],
                    )

            # Q half-0 interleaved with the transposes at 2-s-tile (256 col)
            # granularity: each half-chunk only needs the two s-tiles just
            # transposed, so the PE issues projection work while the next
            # x tiles are still in flight.
            HW2 = NG // 2
            for sg in range(SG):
                for h in range(2):
                    transpose_st(4 * sg + 2 * h)
                    transpose_st(4 * sg + 2 * h + 1)
                    proj_chunk("q", 0, sg, lo=h * HW2, width=HW2)
            for sg in range(SG):
                proj_chunk("q", 1, sg)
            wk_h = [load_w_half("k", 0), load_w_half("k", 1)]
            for half in range(2):
                for sg in range(SG):
                    proj_chunk("k", half, sg)

            # V: [s,u] = sum_d xT[d,s] * Wv[d,u]; bv added in the epilogue
            for ug in range(UG):
                wv_h = load_w_half("v", ug)
                for st in range(ST):
                    pv = psC.tile([P, NG], F32, tag="proj", name="ps_v")
                    for dt in range(DT):
                        nc.tensor.matmul(
                            pv[:],
                            lhsT=xT[:, dt, ts(st, P)],
                            rhs=wv_h[:, dt, :],
                            start=(dt == 0),
                            stop=(dt == DT - 1),
                        )
                    nc.vector.tensor_tensor(
                        v_sb[:, st, ts(ug, NG)],
                        pv[:],
                        bv_bcast[:, ts(ug, NG)],
                        ALU.add,
                    )

        # ---------------- phase D: scores^T + mask + exp ----------------
        # fp8 DoubleRow: each matmul contracts two u-tiles (256 rows).
        et_sb = big.tile([P, KT, S], BF16, tag="slotA", name="et_sb")
        with tc.tile_pool(name="psD", bufs=8, space="PSUM") as psD:
            for kt in range(KT):
                pss = [
                    psD.tile([P, NG], F32, tag="sc", name="ps_sc") for _ in range(QG)
                ]
                for t in range(UT // 2):
                    for qg in range(QG):
                        nc.tensor.matmul(
                            pss[qg][:],
                            lhsT=kt_sb[:, 2 * t : 2 * t + 2, ts(kt, P)],
                            rhs=qt_sb[:, 2 * t : 2 * t + 2, ts(qg, NG)],
                            start=(t == 0),
                            stop=(t == UT // 2 - 1),
                            perf_mode=DR,
                        )
                for qg in range(QG):
                    # scores += c_k * m_q  (rank-1 mask term, on DVE —
                    # GPSIMD cannot access PSUM)
                    nc.vector.scalar_tensor_tensor(
                        pss[qg][:],
                        m_bcast[:, ts(qg, NG)],
                        c_cols[:, kt : kt + 1],
                        pss[qg][:],
                        ALU.mult,
                        ALU.add,
                    )
                    # Et = exp(scores/32); 1/sqrt(U) folded in here
                    nc.scalar.activation(
                        et_sb[:, kt, ts(qg, NG)], pss[qg][:], AF.Exp, scale=1.0 / 32.0
                    )

        # ---------------- phase E: PV(+denom column) + normalize ----------------
        e_off = (0, ESPLIT[0], ESPLIT[0] + ESPLIT[1])
        with tc.tile_pool(name="psE", bufs=6, space="PSUM") as psE:
            for qt in range(KT):
                pc = [
                    psE.tile([P, NG], F32, tag="ctx", name="ps_ctx") for _ in range(3)
                ]
                for kt in range(KT):
                    lhsT = et_sb[:, kt, ts(qt, P)]
                    first, last = kt == 0, kt == KT - 1
                    for j in range(3):
                        nc.tensor.matmul(
                            pc[j][:, 0 : ESPLIT[j]],
                            lhsT=lhsT,
                            rhs=v_sb[:, kt, e_off[j] : e_off[j] + ESPLIT[j]],
                            start=first,
                            stop=last,
                        )
                recip = big.tile([P, 1], F32, tag="kt", name="recip")
                # denominator = ones-column result: last col of chunk 2
                nc.vector.reciprocal(recip[:], pc[2][:, ESPLIT[2] - 1 : ESPLIT[2]])
                # per-chunk normalize + store so the final store starts as
                # early as possible (trims the kernel tail)
                o = big.tile([P, U], F32, tag="qt", name="o_sb")
                for j, w in ((0, ESPLIT[0]), (1, ESPLIT[1]), (2, ESPLIT[2] - 1)):
                    lo = e_off[j]
                    if j == 1:
                        # middle chunk on the (otherwise idle) ACT engine so
                        # the epilogue drains in parallel with the DVE chunks
                        nc.scalar.mul(o[:, lo : lo + w], pc[j][:, 0:w], recip[:])
                    else:
                        nc.vector.tensor_scalar_mul(
                            o[:, lo : lo + w], pc[j][:, 0:w], recip[:]
                        )
                    nc.sync.dma_start(
                        out_d[ts(qt, P), lo : lo + w], o[:, lo : lo + w]
                    )

    free_bv_bcast()
    free_m_bcast()
    free_ident()
    free_rows()
    free_consts()


def _build():
    if "nc" in _cache:
        return _cache["nc"]
    nc = bacc.Bacc("TRN2", target_bir_lowering=False, debug=False, num_devices=NCORES)
    with tile.TileContext(nc) as tc:
        _emit(tc)
    nc.compile()
    _cache["nc"] = nc
    return nc


def kernel(x, mask, Wq, bq, Wk, bk, Wv, bv):
    global last_results
    nc = _build()
    wq = np.ascontiguousarray(Wq, dtype=np.float32)
    wk = np.ascontiguousarray(Wk, dtype=np.float32)
    wv = np.ascontiguousarray(Wv, dtype=np.float32)
    bqr = np.ascontiguousarray(bq, dtype=np.float32).reshape(1, U)
    bkr = np.ascontiguousarray(bk, dtype=np.float32).reshape(1, U)
    bvr = np.ascontiguousarray(bv, dtype=np.float32).reshape(1, U)
    in_maps = []
    for b in range(B):
        in_maps.append(
            {
                "x": np.ascontiguousarray(x[b], dtype=np.float32),
                "mask": np.ascontiguousarray(mask[b], dtype=np.int32).reshape(1, S),
                "wq": wq,
                "wk": wk,
                "wv": wv,
                "bq": bqr,
                "bk": bkr,
                "bv": bvr,
            }
        )
    res = run_bass_kernel_spmd(
        nc,
        in_maps,
        core_ids=list(range(NCORES)),
        trace=bool(int(os.environ.get("KERNEL_TRACE", "0"))),
        tmpdir=os.environ.get("KERNEL_TRACE_DIR"),
    )
    last_results = res
    return np.stack([res.results[b]["out"] for b in range(B)])
